# revision 4
# baseline (speedup 1.0000x reference)
"""Multi-head attention kernel for Trainium2, sharded over 8 NeuronCores.

Problem: q,k,v [2, 4096, 256], 8 heads of d=32.  b*h = 16 head-instances
are sharded 2-per-core (core c -> batch c//4, head-pair c%4).

Per-core algorithm (n=4096, d=32, 2 heads):
  - Load Q,K slabs [4096,64], PE-transpose to qT,kT [64,4096] (d on partitions).
  - Load V with an appended ones-column per 128-row chunk: [128, 33] tiles.
  - For each head, for each q-tile of 1024 columns:
      for each k-chunk of 128 rows:
        S^T[kc, qt] = kT_chunk.T-matmul (contraction d=32) -> PSUM [128,1024]
        P = exp(S^T * 1/sqrt(d))        (ScalarE, PSUM->SBUF)
        O^T[33, qt] += [V|1]_chunk.T @ P (contraction k=128, accumulate PSUM)
      row 32 of O^T is the softmax denominator (ones column).
      PE-transpose O^T back to [128, 33] tiles, multiply by reciprocal
      of column 32, DMA out.
  Softmax max-subtraction is skipped: S ~ N(0,1) after scaling, |S| < ~6.
Matmuls use float32r (TF32-like) via bitcast: 1 cycle/row vs 4 for fp32.
"""

import numpy as np

import concourse.bass as bass
import concourse.mybir as mybir
import concourse.tile as tile
from concourse import bacc, bass_utils
from concourse.masks import make_identity

B, N, C, H, D = 2, 4096, 256, 8, 32
NCORES = 8
HPC = 2                      # heads per core
COLS = HPC * D               # 64 per-core channel columns
P = 128                      # partitions / k-chunk
QTILE = 1024                 # q columns per PSUM/exp tile
NKC = N // P                 # 32 k-chunks
NQT = N // QTILE             # 4 q-tiles per head
SCALE = float(1.0 / np.sqrt(D))
F32 = mybir.dt.float32
F32R = mybir.dt.float32r

_cache = {}


def _emit(tc, nc, q, k, v, out):
    with tc.tile_pool(name="persist", bufs=1) as persist:
        ident = persist.tile([P, P], F32, name="ident")
        make_identity(nc, ident[:])
        qT = persist.tile([COLS, N], F32R, name="qT")
        kT = persist.tile([COLS, N], F32R, name="kT")
        # V with ones column: per head, 32 chunks of [128, 33]
        vsb = persist.tile([P, HPC * NKC * (D + 1)], F32R, name="vsb")

        with tc.tile_pool(name="stage", bufs=1) as stage_pool:
            qstage = stage_pool.tile([P, NKC * COLS], F32, name="qstage")
            kstage = stage_pool.tile([P, NKC * COLS], F32, name="kstage")
            vstage = stage_pool.tile([P, NKC * COLS], F32, name="vstage")

            nc.sync.dma_start(
                qstage[:].rearrange("p (i d) -> p i d", d=COLS),
                q.rearrange("(i p) d -> p i d", p=P),
            )
            nc.sync.dma_start(
                kstage[:].rearrange("p (i d) -> p i d", d=COLS),
                k.rearrange("(i p) d -> p i d", p=P),
            )
            nc.sync.dma_start(
                vstage[:].rearrange("p (i d) -> p i d", d=COLS),
                v.rearrange("(i p) d -> p i d", p=P),
            )
            vv = vsb[:].rearrange("p (hh i e) -> p hh i e", hh=HPC, e=D + 1)
            vst = vstage[:].rearrange("p (i d) -> p i d", d=COLS)
            for hh in range(HPC):
                nc.vector.tensor_copy(
                    vv[:, hh, :, 0:D], vst[:, :, hh * D:(hh + 1) * D]
                )
            onescol = persist.tile([P, HPC * NKC], F32, name="onescol")
            nc.vector.memset(onescol[:], 1.0)
            nc.vector.tensor_copy(
                vv[:, :, :, D],
                onescol[:].rearrange("p (hh i) -> p hh i", hh=HPC),
            )

            with tc.tile_pool(name="tp", bufs=4, space="PSUM") as tp:
                for i in range(NKC):
                    for st, dstT in ((qstage, qT), (kstage, kT)):
                        pt = tp.tile([COLS, P], F32, tag="pt")
                        nc.tensor.transpose(
                            pt[:], st[:, i * COLS:(i + 1) * COLS], ident[:]
                        )
                        nc.vector.tensor_copy(dstT[:, i * P:(i + 1) * P], pt[:])

        with (
            tc.tile_pool(name="ps", bufs=2, space="PSUM") as ps_pool,
            tc.tile_pool(name="po", bufs=1, space="PSUM") as po_pool,
            tc.tile_pool(name="pt2", bufs=2, space="PSUM") as pt2_pool,
            tc.tile_pool(name="pexp", bufs=3) as pexp_pool,
            tc.tile_pool(name="osb", bufs=2) as osb_pool,
            tc.tile_pool(name="rec", bufs=3) as rec_pool,
            tc.tile_pool(name="outsb", bufs=3) as outsb_pool,
        ):
            for hh in range(HPC):
                hp = slice(D * hh, D * (hh + 1))      # partition rows of qT/kT
                vbase = hh * NKC * (D + 1)
                for qt in range(NQT):
                    q0 = qt * QTILE
                    po = po_pool.tile([D + 1, QTILE], F32, tag="po")
                    for kc in range(NKC):
                        ps = ps_pool.tile([P, QTILE], F32, tag="ps")
                        for m in range(QTILE // 512):
                            nc.tensor.matmul(
                                ps[:, m * 512:(m + 1) * 512],
                                lhsT=kT[hp, kc * P:(kc + 1) * P],
                                rhs=qT[hp, q0 + m * 512:q0 + (m + 1) * 512],
                                start=True,
                                stop=True,
                            )
                        pexp = pexp_pool.tile([P, QTILE], F32R, tag="pexp")
                        nc.scalar.activation(
                            pexp[:], ps[:], mybir.ActivationFunctionType.Exp,
                            scale=SCALE,
                        )
                        vch = vsb[:, vbase + kc * (D + 1):vbase + (kc + 1) * (D + 1)]
                        for m in range(QTILE // 512):
                            nc.tensor.matmul(
                                po[:, m * 512:(m + 1) * 512],
                                lhsT=vch,
                                rhs=pexp[:, m * 512:(m + 1) * 512],
                                start=(kc == 0),
                                stop=(kc == NKC - 1),
                            )
                    osb = osb_pool.tile([D + 1, QTILE], F32, tag="osb")
                    nc.vector.tensor_copy(osb[:], po[:])
                    for j in range(QTILE // P):
                        pt2 = pt2_pool.tile([P, D + 1], F32, tag="pt2")
                        nc.tensor.transpose(
                            pt2[:], osb[:, j * P:(j + 1) * P],
                            ident[0:D + 1, 0:D + 1],
                        )
                        rec = rec_pool.tile([P, 1], F32, tag="rec")
                        nc.vector.reciprocal(rec[:], pt2[:, D:D + 1])
                        outsb = outsb_pool.tile([P, D], F32, tag="outsb")
                        nc.vector.tensor_scalar_mul(outsb[:], pt2[:, 0:D], rec[:])
                        nc.sync.dma_start(
                            out[q0 + j * P:q0 + (j + 1) * P, hp], outsb[:]
                        )


def _build():
    if "nc" in _cache:
        return _cache["nc"]
    nc = bacc.Bacc(
        "TRN2",
        target_bir_lowering=False,
        debug=False,
        enable_asserts=False,
        num_devices=NCORES,
    )
    q = nc.dram_tensor("q", [N, COLS], F32, kind="ExternalInput").ap()
    k = nc.dram_tensor("k", [N, COLS], F32, kind="ExternalInput").ap()
    v = nc.dram_tensor("v", [N, COLS], F32, kind="ExternalInput").ap()
    out = nc.dram_tensor("out", [N, COLS], F32, kind="ExternalOutput").ap()
    with tile.TileContext(nc) as tc:
        _emit(tc, nc, q, k, v, out)
    nc.compile()
    _cache["nc"] = nc
    return nc


def _in_maps(q, k, v):
    maps = []
    for c in range(NCORES):
        b, hp = divmod(c, 4)
        cs = slice(hp * COLS, (hp + 1) * COLS)
        maps.append({
            "q": np.ascontiguousarray(q[b, :, cs], dtype=np.float32),
            "k": np.ascontiguousarray(k[b, :, cs], dtype=np.float32),
            "v": np.ascontiguousarray(v[b, :, cs], dtype=np.float32),
        })
    return maps


def _assemble(results):
    out = np.empty((B, N, C), np.float32)
    for c in range(NCORES):
        b, hp = divmod(c, 4)
        out[b, :, hp * COLS:(hp + 1) * COLS] = results[c]["out"]
    return out


def kernel(q, k, v):
    nc = _build()
    res = bass_utils.run_bass_kernel_spmd(
        nc, _in_maps(q, k, v), core_ids=list(range(NCORES))
    )
    return _assemble(res.results)


# revision 5
# speedup vs baseline: 11.5110x; 11.5110x over previous
"""Multi-head attention kernel for Trainium2, sharded over 8 NeuronCores.

Problem: q,k,v [2, 4096, 256], 8 heads of d=32.  b*h = 16 head-instances
are sharded 2-per-core (core c -> batch c//4, head-pair c%4).

Per-core algorithm (n=4096, d=32, 2 heads):
  - Load Q,K slabs [4096,64], PE-transpose to qT,kT [64,4096] (d on partitions).
  - Load V with an appended ones-column per 128-row chunk: [128, 33] tiles.
  - For each head, for each q-tile of 1024 columns:
      for each k-chunk of 128 rows:
        S^T[kc, qt] = kT_chunk.T-matmul (contraction d=32) -> PSUM [128,1024]
        P = exp(S^T * 1/sqrt(d))        (ScalarE, PSUM->SBUF)
        O^T[33, qt] += [V|1]_chunk.T @ P (contraction k=128, accumulate PSUM)
      row 32 of O^T is the softmax denominator (ones column).
      PE-transpose O^T back to [128, 33] tiles, multiply by reciprocal
      of column 32, DMA out.
  Softmax max-subtraction is skipped: S ~ N(0,1) after scaling, |S| < ~6.
Matmuls use float32r (TF32-like) via bitcast: 1 cycle/row vs 4 for fp32.
"""

import numpy as np

import concourse.bass as bass
import concourse.mybir as mybir
import concourse.tile as tile
from concourse import bacc, bass_utils
from concourse.masks import make_identity

B, N, C, H, D = 2, 4096, 256, 8, 32
NCORES = 8
HPC = 2                      # heads per core
COLS = HPC * D               # 64 per-core channel columns
P = 128                      # partitions / k-chunk
QTILE = 1024                 # q columns per PSUM/exp tile
NKC = N // P                 # 32 k-chunks
NQT = N // QTILE             # 4 q-tiles per head
SCALE = float(1.0 / np.sqrt(D))
F32 = mybir.dt.float32
F32R = mybir.dt.float32r

_cache = {}


def _emit(tc, nc, q, k, v, out):
    with tc.tile_pool(name="persist", bufs=1) as persist:
        ident = persist.tile([P, P], F32, name="ident")
        make_identity(nc, ident[:])
        qT = persist.tile([COLS, N], F32R, name="qT")
        kT = persist.tile([COLS, N], F32R, name="kT")
        # V with ones column: per head, 32 chunks of [128, 33]
        vsb = persist.tile([P, HPC * NKC * (D + 1)], F32R, name="vsb")

        with tc.tile_pool(name="stage", bufs=1) as stage_pool:
            qstage = stage_pool.tile([P, NKC * COLS], F32, name="qstage")
            kstage = stage_pool.tile([P, NKC * COLS], F32, name="kstage")
            vstage = stage_pool.tile([P, NKC * COLS], F32, name="vstage")

            nc.sync.dma_start(
                qstage[:].rearrange("p (i d) -> p i d", d=COLS),
                q.rearrange("(i p) d -> p i d", p=P),
            )
            nc.sync.dma_start(
                kstage[:].rearrange("p (i d) -> p i d", d=COLS),
                k.rearrange("(i p) d -> p i d", p=P),
            )
            nc.sync.dma_start(
                vstage[:].rearrange("p (i d) -> p i d", d=COLS),
                v.rearrange("(i p) d -> p i d", p=P),
            )
            vv = vsb[:].rearrange("p (hh i e) -> p hh i e", hh=HPC, e=D + 1)
            vst = vstage[:].rearrange("p (i d) -> p i d", d=COLS)
            for hh in range(HPC):
                nc.vector.tensor_copy(
                    vv[:, hh, :, 0:D], vst[:, :, hh * D:(hh + 1) * D]
                )
            onescol = persist.tile([P, HPC * NKC], F32, name="onescol")
            nc.vector.memset(onescol[:], 1.0)
            nc.vector.tensor_copy(
                vv[:, :, :, D],
                onescol[:].rearrange("p (hh i) -> p hh i", hh=HPC),
            )

            with tc.tile_pool(name="tp", bufs=4, space="PSUM") as tp:
                for i in range(NKC):
                    for st, dstT in ((qstage, qT), (kstage, kT)):
                        pt = tp.tile([COLS, P], F32, tag="pt")
                        nc.tensor.transpose(
                            pt[:], st[:, i * COLS:(i + 1) * COLS], ident[:]
                        )
                        nc.vector.tensor_copy(dstT[:, i * P:(i + 1) * P], pt[:])

        with (
            tc.tile_pool(name="ps", bufs=2, space="PSUM") as ps_pool,
            tc.tile_pool(name="po", bufs=1, space="PSUM") as po_pool,
            tc.tile_pool(name="pt2", bufs=2, space="PSUM") as pt2_pool,
            tc.tile_pool(name="pexp", bufs=3) as pexp_pool,
            tc.tile_pool(name="osb", bufs=2) as osb_pool,
            tc.tile_pool(name="rec", bufs=3) as rec_pool,
            tc.tile_pool(name="outsb", bufs=3) as outsb_pool,
        ):
            for hh in range(HPC):
                hp = slice(D * hh, D * (hh + 1))      # partition rows of qT/kT
                vbase = hh * NKC * (D + 1)
                for qt in range(NQT):
                    q0 = qt * QTILE
                    po = po_pool.tile([D + 1, QTILE], F32, tag="po")
                    for kc in range(NKC):
                        ps = ps_pool.tile([P, QTILE], F32, tag="ps")
                        for m in range(QTILE // 512):
                            nc.tensor.matmul(
                                ps[:, m * 512:(m + 1) * 512],
                                lhsT=kT[hp, kc * P:(kc + 1) * P],
                                rhs=qT[hp, q0 + m * 512:q0 + (m + 1) * 512],
                                start=True,
                                stop=True,
                            )
                        pexp = pexp_pool.tile([P, QTILE], F32R, tag="pexp")
                        nc.scalar.activation(
                            pexp[:], ps[:], mybir.ActivationFunctionType.Exp,
                            scale=SCALE,
                        )
                        vch = vsb[:, vbase + kc * (D + 1):vbase + (kc + 1) * (D + 1)]
                        for m in range(QTILE // 512):
                            nc.tensor.matmul(
                                po[:, m * 512:(m + 1) * 512],
                                lhsT=vch,
                                rhs=pexp[:, m * 512:(m + 1) * 512],
                                start=(kc == 0),
                                stop=(kc == NKC - 1),
                            )
                    osb = osb_pool.tile([D + 1, QTILE], F32, tag="osb")
                    nc.vector.tensor_copy(osb[:], po[:])
                    for j in range(QTILE // P):
                        pt2 = pt2_pool.tile([P, D + 1], F32, tag="pt2")
                        nc.tensor.transpose(
                            pt2[:], osb[:, j * P:(j + 1) * P],
                            ident[0:D + 1, 0:D + 1],
                        )
                        rec = rec_pool.tile([P, 1], F32, tag="rec")
                        nc.vector.reciprocal(rec[:], pt2[:, D:D + 1])
                        outsb = outsb_pool.tile([P, D], F32, tag="outsb")
                        nc.vector.tensor_scalar_mul(outsb[:], pt2[:, 0:D], rec[:])
                        nc.sync.dma_start(
                            out[q0 + j * P:q0 + (j + 1) * P, hp], outsb[:]
                        )


def _build(loop=0):
    """loop=0: production build.  loop>=1: body wrapped in an on-device
    For_i repeat loop (timing-only builds)."""
    key = ("nc", loop)
    if key in _cache:
        return _cache[key]
    nc = bacc.Bacc(
        "TRN2",
        target_bir_lowering=False,
        debug=False,
        enable_asserts=False,
        num_devices=NCORES,
    )
    q = nc.dram_tensor("q", [N, COLS], F32, kind="ExternalInput").ap()
    k = nc.dram_tensor("k", [N, COLS], F32, kind="ExternalInput").ap()
    v = nc.dram_tensor("v", [N, COLS], F32, kind="ExternalInput").ap()
    out = nc.dram_tensor("out", [N, COLS], F32, kind="ExternalOutput").ap()
    with tile.TileContext(nc) as tc:
        if loop:
            with tc.For_i(0, loop, 1):
                _emit(tc, nc, q, k, v, out)
        else:
            _emit(tc, nc, q, k, v, out)
    nc.compile()
    _cache[key] = nc
    return nc


def _in_maps(q, k, v):
    maps = []
    for c in range(NCORES):
        b, hp = divmod(c, 4)
        cs = slice(hp * COLS, (hp + 1) * COLS)
        maps.append({
            "q": np.ascontiguousarray(q[b, :, cs], dtype=np.float32),
            "k": np.ascontiguousarray(k[b, :, cs], dtype=np.float32),
            "v": np.ascontiguousarray(v[b, :, cs], dtype=np.float32),
        })
    return maps


def _assemble(results):
    out = np.empty((B, N, C), np.float32)
    for c in range(NCORES):
        b, hp = divmod(c, 4)
        out[b, :, hp * COLS:(hp + 1) * COLS] = results[c]["out"]
    return out


def kernel(q, k, v):
    nc = _build()
    res = bass_utils.run_bass_kernel_spmd(
        nc, _in_maps(q, k, v), core_ids=list(range(NCORES))
    )
    return _assemble(res.results)
